# revision 3
# baseline (speedup 1.0000x reference)
"""Causal self-attention Trainium2 kernel (8 NeuronCores).

Sharding (Megatron-style, per sharding_hint):
  core c -> batch b = c//2, head-group g = c%2 (8 of 16 heads).
  W_q/W_k/W_v column-sliced per head group; W_o row-sliced; host sums the
  two partial outputs per batch (tensor-parallel reduce) and adds b_o.

Per-core kernel (all matmuls bf16 with fp32 PSUM accumulation):
  xT    [1024, 2048]  x[b] transposed (d_emb on partitions)
  wqkv  [1024, 1536]  [Wq_g | Wk_g | Wv_g]
  wo    [512, 1024]   W_o rows for this head group
  out   [2048, 1024]  fp32 partial (no bias)

Layouts: qT/kT stored [head_dim, n] so score matmuls contract over the
64-dim head axis; scores are computed TRANSPOSED ([k, q]) so the exp'd
weights can feed the ctx matmul directly as the moving operand; v is kept
[n, head_dim] and augmented with a ones-column so a single matmul yields
both ctx^T and the softmax denominators broadcast across 64 partitions.
"""

import os
import sys

import numpy as np

sys.path.insert(0, "/opt/trn_rl_repo")

import ml_dtypes

BF16 = ml_dtypes.bfloat16

D_EMB = 1024
N_SEQ = 2048
N_HEADS_CORE = 8  # heads per core
HD = 64  # head dim
KT = D_EMB // 128  # 8 k-tiles over d_emb
PT = 4  # partition tiles over the 512 per-core head dims
NT = N_SEQ // 128  # 16 n-tiles
QC = N_SEQ // 512  # 4 query chunks of 512
SCALE = 1.0 / np.sqrt(HD)

_CACHE = {}


def _build_module():
    import concourse.bacc as bacc
    import concourse.mybir as mybir
    import concourse.tile as tile

    f32 = mybir.dt.float32
    bf16 = mybir.dt.bfloat16

    nc = bacc.Bacc()
    xT_d = nc.dram_tensor("xT", [D_EMB, N_SEQ], bf16, kind="ExternalInput")
    wqkv_d = nc.dram_tensor("wqkv", [D_EMB, 1536], bf16, kind="ExternalInput")
    wo_d = nc.dram_tensor("wo", [512, D_EMB], bf16, kind="ExternalInput")
    out_d = nc.dram_tensor("out", [N_SEQ, D_EMB], f32, kind="ExternalOutput")

    with tile.TileContext(nc) as tc:
        with (
            tc.tile_pool(name="persist", bufs=1) as persist,
            tc.tile_pool(name="expp", bufs=4) as expp,
            tc.tile_pool(name="rpool", bufs=4) as rpool,
            tc.tile_pool(name="outp", bufs=3) as outp,
        ):
            # ---- persistent SBUF tensors ----
            xt_sb = [
                persist.tile([128, N_SEQ], bf16, name=f"xt{k}", tag=f"xt{k}")
                for k in range(KT)
            ]
            wqkv_sb = [
                persist.tile([128, 1536], bf16, name=f"wqkv{k}", tag=f"wqkv{k}")
                for k in range(KT)
            ]
            wo_sb = [
                persist.tile([128, D_EMB], bf16, name=f"wo{p}", tag=f"wo{p}")
                for p in range(PT)
            ]
            qt_sb = [
                persist.tile([128, N_SEQ], bf16, name=f"qt{p}", tag=f"qt{p}")
                for p in range(PT)
            ]
            kt_sb = [
                persist.tile([128, N_SEQ], bf16, name=f"kt{p}", tag=f"kt{p}")
                for p in range(PT)
            ]
            ctxt_sb = [
                persist.tile([128, N_SEQ], bf16, name=f"ctxt{p}", tag=f"ctxt{p}")
                for p in range(PT)
            ]
            # v augmented with ones column-block: [128, 128] per (ntile, head):
            # cols 0:64 = v_h, cols 64:128 = 1.0 (softmax denominator trick)
            vaug_sb = [
                [
                    persist.tile(
                        [128, 128], bf16, name=f"vaug{nt}_{h}", tag=f"vaug{nt}_{h}"
                    )
                    for h in range(N_HEADS_CORE)
                ]
                for nt in range(NT)
            ]
            mask_sb = [
                persist.tile([128, 512], bf16, name=f"mask{j}", tag=f"mask{j}")
                for j in range(4)
            ]

            # ---- constants: ones columns + causal masks (gpsimd, no deps) ----
            for nt in range(NT):
                for h in range(N_HEADS_CORE):
                    nc.gpsimd.memset(vaug_sb[nt][h][:, 64:128], 1.0)
            for j in range(4):
                # mask_j[k_local, q_local] = 1.0 if q_local - k_local - 128*j >= 0 else 0
                nc.gpsimd.memset(mask_sb[j][:], 1.0)
                nc.gpsimd.affine_select(
                    out=mask_sb[j][:],
                    in_=mask_sb[j][:],
                    compare_op=mybir.AluOpType.is_ge,
                    fill=0.0,
                    base=-128 * j,
                    pattern=[[1, 512]],
                    channel_multiplier=-1,
                )

            # ---- input DMA ----
            for k in range(KT):
                nc.sync.dma_start(out=xt_sb[k][:], in_=xT_d[k * 128 : (k + 1) * 128, :])
                nc.sync.dma_start(
                    out=wqkv_sb[k][:], in_=wqkv_d[k * 128 : (k + 1) * 128, :]
                )
            for p in range(PT):
                nc.sync.dma_start(out=wo_sb[p][:], in_=wo_d[p * 128 : (p + 1) * 128, :])

            with tc.tile_pool(name="psq", bufs=3, space="PSUM") as psq_pool:
                # ---- v = x @ Wv  ([n, 64] per head, + ones) ----
                for nt in range(NT):
                    psv = psq_pool.tile([128, 512], f32, name=f"psv{nt}", tag="psv")
                    for k in range(KT):
                        nc.tensor.matmul(
                            psv[:],
                            lhsT=xt_sb[k][:, nt * 128 : (nt + 1) * 128],
                            rhs=wqkv_sb[k][:, 1024:1536],
                            start=(k == 0),
                            stop=(k == KT - 1),
                        )
                    for h in range(N_HEADS_CORE):
                        nc.vector.tensor_copy(
                            vaug_sb[nt][h][:, 0:64], psv[:, h * 64 : (h + 1) * 64]
                        )

                # ---- qT, kT = (x @ Wq)^T, (x @ Wk)^T  [hd, n] layout ----
                for p in range(PT):
                    for qn in range(QC):
                        nsl = slice(qn * 512, (qn + 1) * 512)
                        for which, dst in ((0, qt_sb), (1, kt_sb)):
                            ps = psq_pool.tile(
                                [128, 512], f32, name=f"psqk{p}_{qn}_{which}", tag="psqk"
                            )
                            col0 = which * 512 + p * 128
                            for k in range(KT):
                                nc.tensor.matmul(
                                    ps[:],
                                    lhsT=wqkv_sb[k][:, col0 : col0 + 128],
                                    rhs=xt_sb[k][:, nsl],
                                    start=(k == 0),
                                    stop=(k == KT - 1),
                                )
                            nc.vector.tensor_copy(dst[p][:, nsl], ps[:])

            # ---- attention ----
            with (
                tc.tile_pool(name="pssc", bufs=2, space="PSUM") as pssc_pool,
                tc.tile_pool(name="psctx", bufs=4, space="PSUM") as psctx_pool,
            ):
                for p in range(PT):
                    for qc in range(QC):
                        qsl = slice(qc * 512, (qc + 1) * 512)
                        nk = 4 * qc + 4  # causal: k-tiles 0..nk-1
                        ngroups = nk // 2
                        ctx_ps = [
                            psctx_pool.tile(
                                [128, 512], f32, name=f"ctx{p}_{qc}_{h2}", tag="ctx"
                            )
                            for h2 in range(2)
                        ]
                        for gi in range(ngroups):
                            for h2 in range(2):
                                hb = h2 * 64
                                h = 2 * p + h2
                                ps = pssc_pool.tile(
                                    [128, 1024],
                                    f32,
                                    name=f"sc{p}_{qc}_{gi}_{h2}",
                                    tag="sc",
                                )
                                for j in range(2):
                                    ki = 2 * gi + j
                                    nc.tensor.matmul(
                                        ps[:, j * 512 : (j + 1) * 512],
                                        lhsT=kt_sb[p][
                                            hb : hb + 64, ki * 128 : (ki + 1) * 128
                                        ],
                                        rhs=qt_sb[p][hb : hb + 64, qsl],
                                        start=True,
                                        stop=True,
                                    )
                                ex = expp.tile(
                                    [128, 1024],
                                    bf16,
                                    name=f"ex{p}_{qc}_{gi}_{h2}",
                                    tag="ex",
                                )
                                nc.scalar.activation(
                                    ex[:],
                                    ps[:],
                                    mybir.ActivationFunctionType.Exp,
                                    scale=float(SCALE),
                                )
                                for j in range(2):
                                    ki = 2 * gi + j
                                    jj = ki - 4 * qc
                                    if jj >= 0:  # diagonal tile: causal mask
                                        nc.vector.tensor_mul(
                                            ex[:, j * 512 : (j + 1) * 512],
                                            ex[:, j * 512 : (j + 1) * 512],
                                            mask_sb[jj][:],
                                        )
                                for j in range(2):
                                    ki = 2 * gi + j
                                    nc.tensor.matmul(
                                        ctx_ps[h2][:],
                                        lhsT=vaug_sb[ki][h][:],
                                        rhs=ex[:, j * 512 : (j + 1) * 512],
                                        start=(ki == 0),
                                        stop=(ki == nk - 1),
                                    )
                        for h2 in range(2):
                            rec = rpool.tile(
                                [64, 512], f32, name=f"rec{p}_{qc}_{h2}", tag="rec"
                            )
                            nc.vector.reciprocal(rec[:], ctx_ps[h2][64:128, :])
                            nc.vector.tensor_mul(
                                ctxt_sb[p][h2 * 64 : h2 * 64 + 64, qsl],
                                ctx_ps[h2][0:64, :],
                                rec[:],
                            )

            # ---- out = ctx @ Wo (partial; host adds the other half + bias) ----
            with tc.tile_pool(name="psout", bufs=3, space="PSUM") as psout_pool:
                for nt in range(NT):
                    for dh in range(2):
                        pso = psout_pool.tile(
                            [128, 512], f32, name=f"pso{nt}_{dh}", tag="pso"
                        )
                        for p in range(PT):
                            nc.tensor.matmul(
                                pso[:],
                                lhsT=ctxt_sb[p][:, nt * 128 : (nt + 1) * 128],
                                rhs=wo_sb[p][:, dh * 512 : (dh + 1) * 512],
                                start=(p == 0),
                                stop=(p == PT - 1),
                            )
                        osb = outp.tile([128, 512], f32, name=f"osb{nt}_{dh}", tag="osb")
                        nc.vector.tensor_copy(osb[:], pso[:])
                        nc.sync.dma_start(
                            out=out_d[
                                nt * 128 : (nt + 1) * 128, dh * 512 : (dh + 1) * 512
                            ],
                            in_=osb[:],
                        )

    if not nc.is_finalized():
        nc.finalize()
    return nc


def _get_module():
    if "nc" not in _CACHE:
        _CACHE["nc"] = _build_module()
    return _CACHE["nc"]


def kernel(x, W_q, W_k, W_v, W_o, b_o, _trace=False):
    from concourse.bass_utils import run_bass_kernel_spmd

    nc = _get_module()

    in_maps = []
    for c in range(8):
        b, g = c // 2, c % 2
        gs = slice(g * 512, (g + 1) * 512)
        xT = np.ascontiguousarray(x[b].T).astype(BF16)
        wqkv = np.concatenate(
            [W_q[:, gs], W_k[:, gs], W_v[:, gs]], axis=1
        ).astype(BF16)
        wo = np.ascontiguousarray(W_o[gs, :]).astype(BF16)
        in_maps.append({"xT": xT, "wqkv": wqkv, "wo": wo})

    res = run_bass_kernel_spmd(nc, in_maps, core_ids=list(range(8)), trace=_trace)
    if _trace:
        _CACHE["last_results"] = res

    out = np.empty((4, N_SEQ, D_EMB), np.float32)
    for b in range(4):
        out[b] = (
            res.results[2 * b]["out"]
            + res.results[2 * b + 1]["out"]
            + b_o[None, :].astype(np.float32)
        )
    return out


# revision 9
# speedup vs baseline: 62.1866x; 62.1866x over previous
"""Causal self-attention Trainium2 kernel (8 NeuronCores).

Sharding (Megatron-style, per sharding_hint):
  core c -> batch b = c//2, head-group g = c%2 (8 of 16 heads).
  W_q/W_k/W_v column-sliced per head group; W_o row-sliced; host sums the
  two partial outputs per batch (tensor-parallel reduce) and adds b_o.

Per-core kernel (all matmuls bf16 with fp32 PSUM accumulation):
  xT    [1024, 2048]  x[b] transposed (d_emb on partitions)
  wqkv  [1024, 1536]  [Wq_g | Wk_g | Wv_g]
  wo    [512, 1024]   W_o rows for this head group
  out   [2048, 1024]  fp32 partial (no bias)

Layouts: qT/kT stored [head_dim, n] so score matmuls contract over the
64-dim head axis (two heads share the 128 partitions -> concurrent
row-group matmuls on the PE); scores are computed TRANSPOSED ([k, q]) so
the exp'd weights feed the ctx matmul directly as the moving operand; v is
kept [n, head_dim] with an appended ones-block so a single matmul yields
both ctx^T and the softmax denominators broadcast across 64 partitions
(2-block weight access pattern selects head's v columns + shared ones).

Causal handling: k-tiles strictly above the diagonal are skipped; on
diagonal sub-tiles the fully-masked query prefix is simply never computed
(scores and ctx matmuls trim their moving operand to q >= k-block start),
and only the 128x128 diagonal block gets a triangular bf16 multiply.
"""

import sys

import numpy as np

sys.path.insert(0, "/opt/trn_rl_repo")

import ml_dtypes

BF16 = ml_dtypes.bfloat16

D_EMB = 1024
N_SEQ = 2048
N_HEADS_CORE = 8  # heads per core
HD = 64  # head dim
KT = D_EMB // 128  # 8 k-tiles over d_emb
PT = 4  # partition tiles over the 512 per-core head dims
NT = N_SEQ // 128  # 16 n-tiles
QC = N_SEQ // 512  # 4 query chunks of 512
SCALE = 1.0 / np.sqrt(HD)

_CACHE = {}


def _build_module():
    import concourse.bacc as bacc
    import concourse.bass as bass
    import concourse.mybir as mybir
    import concourse.tile as tile

    f32 = mybir.dt.float32
    bf16 = mybir.dt.bfloat16

    nc = bacc.Bacc()
    xT_d = nc.dram_tensor("xT", [D_EMB, N_SEQ], bf16, kind="ExternalInput")
    wqkv_d = nc.dram_tensor("wqkv", [D_EMB, 1536], bf16, kind="ExternalInput")
    wo_d = nc.dram_tensor("wo", [512, D_EMB], bf16, kind="ExternalInput")
    out_d = nc.dram_tensor("out", [N_SEQ, D_EMB], f32, kind="ExternalOutput")

    with tile.TileContext(nc) as tc:
        with tc.tile_pool(name="persist", bufs=1) as persist:
            # ---- persistent SBUF tensors ----
            wo_sb = [
                persist.tile([128, D_EMB], bf16, name=f"wo{p}", tag=f"wo{p}")
                for p in range(PT)
            ]
            qt_sb = [
                persist.tile([128, N_SEQ], bf16, name=f"qt{p}", tag=f"qt{p}")
                for p in range(PT)
            ]
            kt_sb = [
                persist.tile([128, N_SEQ], bf16, name=f"kt{p}", tag=f"kt{p}")
                for p in range(PT)
            ]
            ctxt_sb = [
                persist.tile([128, N_SEQ], bf16, name=f"ctxt{p}", tag=f"ctxt{p}")
                for p in range(PT)
            ]
            # v per n-tile [128, 1024]: head h -> cols h*128:h*128+64 = v_h,
            # cols h*128+64:h*128+128 = 1.0 (softmax denominator ones-trick)
            v_sb = [
                persist.tile([128, 1024], bf16, name=f"v{nt}", tag=f"v{nt}")
                for nt in range(NT)
            ]
            tri_sb = persist.tile([128, 128], bf16, name="tri", tag="tri")

            def vaug_ap(nt, h):
                return v_sb[nt][:, h * 128 : (h + 1) * 128]

            # ---- constants (gpsimd, no deps) ----
            for nt in range(NT):
                ones_view = v_sb[nt].rearrange("p (h c) -> p h c", h=N_HEADS_CORE)
                nc.gpsimd.memset(ones_view[:, :, 64:128], 1.0)
            # tri[k_local, q_local] = 1.0 if q_local >= k_local else 0
            nc.gpsimd.memset(tri_sb[:], 1.0)
            nc.gpsimd.affine_select(
                out=tri_sb[:],
                in_=tri_sb[:],
                compare_op=mybir.AluOpType.is_ge,
                fill=0.0,
                base=0,
                pattern=[[1, 128]],
                channel_multiplier=-1,
            )

            with tc.tile_pool(name="xw", bufs=1) as xw_pool:
                xt_sb = [
                    xw_pool.tile([128, N_SEQ], bf16, name=f"xt{k}", tag=f"xt{k}")
                    for k in range(KT)
                ]
                wqkv_sb = [
                    xw_pool.tile([128, 1536], bf16, name=f"wqkv{k}", tag=f"wqkv{k}")
                    for k in range(KT)
                ]

                # ---- input DMA ----
                for k in range(KT):
                    nc.sync.dma_start(
                        out=xt_sb[k][:], in_=xT_d[k * 128 : (k + 1) * 128, :]
                    )
                    nc.sync.dma_start(
                        out=wqkv_sb[k][:], in_=wqkv_d[k * 128 : (k + 1) * 128, :]
                    )
                for p in range(PT):
                    nc.sync.dma_start(
                        out=wo_sb[p][:], in_=wo_d[p * 128 : (p + 1) * 128, :]
                    )

                with tc.tile_pool(name="psq", bufs=3, space="PSUM") as psq_pool:
                    # ---- v = x @ Wv  ([n, 512] per n-tile) ----
                    for nt in range(NT):
                        psv = psq_pool.tile([128, 512], f32, name=f"psv{nt}", tag="psv")
                        for k in range(KT):
                            nc.tensor.matmul(
                                psv[:],
                                lhsT=xt_sb[k][:, nt * 128 : (nt + 1) * 128],
                                rhs=wqkv_sb[k][:, 1024:1536],
                                start=(k == 0),
                                stop=(k == KT - 1),
                            )
                        v_view = v_sb[nt].rearrange(
                            "p (h c) -> p h c", h=N_HEADS_CORE
                        )
                        nc.vector.tensor_copy(
                            v_view[:, :, 0:64],
                            psv.rearrange("p (h c) -> p h c", h=N_HEADS_CORE),
                        )

                    # ---- qT, kT = (x @ Wq)^T, (x @ Wk)^T  [hd, n] layout ----
                    for p in range(PT):
                        for qn in range(QC):
                            nsl = slice(qn * 512, (qn + 1) * 512)
                            for which, dst in ((0, qt_sb), (1, kt_sb)):
                                ps = psq_pool.tile(
                                    [128, 512],
                                    f32,
                                    name=f"psqk{p}_{qn}_{which}",
                                    tag="psqk",
                                )
                                col0 = which * 512 + p * 128
                                for k in range(KT):
                                    nc.tensor.matmul(
                                        ps[:],
                                        lhsT=wqkv_sb[k][:, col0 : col0 + 128],
                                        rhs=xt_sb[k][:, nsl],
                                        start=(k == 0),
                                        stop=(k == KT - 1),
                                    )
                                nc.vector.tensor_copy(dst[p][:, nsl], ps[:])

            # ---- attention ----
            with (
                tc.tile_pool(name="expp", bufs=4) as expp,
                tc.tile_pool(name="rpool", bufs=4) as rpool,
                tc.tile_pool(name="pssc", bufs=2, space="PSUM") as pssc_pool,
                tc.tile_pool(name="psctx", bufs=2, space="PSUM") as psctx_pool,
            ):
                for p in range(PT):
                    for qc in range(QC):
                        q0 = qc * 512
                        nk = 4 * qc + 4  # causal: k-tiles 0..nk-1
                        ngroups = nk // 2
                        # both heads' ctx in one 2-bank tile: h2 -> cols h2*512
                        ctx_ps = psctx_pool.tile(
                            [128, 1024], f32, name=f"ctx{p}_{qc}", tag="ctx"
                        )
                        for gi in range(ngroups):
                            for h2 in range(2):
                                hb = h2 * 64
                                h = 2 * p + h2
                                ps = pssc_pool.tile(
                                    [128, 1024],
                                    f32,
                                    name=f"sc{p}_{qc}_{gi}_{h2}",
                                    tag="sc",
                                )
                                ex = expp.tile(
                                    [128, 1024],
                                    bf16,
                                    name=f"ex{p}_{qc}_{gi}_{h2}",
                                    tag="ex",
                                )
                                for j in range(2):
                                    ki = 2 * gi + j
                                    jj = ki - 4 * qc  # >=0 on diagonal sub-tiles
                                    t0 = max(0, 128 * jj)  # masked-prefix trim
                                    nc.tensor.matmul(
                                        ps[:, j * 512 + t0 : (j + 1) * 512],
                                        lhsT=kt_sb[p][
                                            hb : hb + 64, ki * 128 : (ki + 1) * 128
                                        ],
                                        rhs=qt_sb[p][hb : hb + 64, q0 + t0 : q0 + 512],
                                        start=True,
                                        stop=True,
                                    )
                                nc.scalar.activation(
                                    ex[:],
                                    ps[:],
                                    mybir.ActivationFunctionType.Exp,
                                    scale=float(SCALE),
                                )
                                for j in range(2):
                                    ki = 2 * gi + j
                                    jj = ki - 4 * qc
                                    if jj >= 0:  # triangular block on the diagonal
                                        blk = slice(j * 512 + 128 * jj,
                                                    j * 512 + 128 * jj + 128)
                                        nc.vector.tensor_mul(
                                            ex[:, blk], ex[:, blk], tri_sb[:]
                                        )
                                for j in range(2):
                                    ki = 2 * gi + j
                                    jj = ki - 4 * qc
                                    t0 = max(0, 128 * jj)
                                    nc.tensor.matmul(
                                        ctx_ps[:, h2 * 512 + t0 : (h2 + 1) * 512],
                                        lhsT=vaug_ap(ki, h),
                                        rhs=ex[:, j * 512 + t0 : (j + 1) * 512],
                                        start=(ki == 0),
                                        stop=(ki == nk - 1),
                                    )
                        rec = rpool.tile(
                            [64, 1024], f32, name=f"rec{p}_{qc}", tag="rec"
                        )
                        nc.vector.reciprocal(rec[:], ctx_ps[64:128, :])
                        for h2 in range(2):
                            nc.vector.tensor_mul(
                                ctxt_sb[p][h2 * 64 : h2 * 64 + 64, q0 : q0 + 512],
                                ctx_ps[0:64, h2 * 512 : (h2 + 1) * 512],
                                rec[:, h2 * 512 : (h2 + 1) * 512],
                            )

            # ---- out = ctx @ Wo (partial; host adds the other half + bias) ----
            with (
                tc.tile_pool(name="outp", bufs=3) as outp,
                tc.tile_pool(name="psout", bufs=2, space="PSUM") as psout_pool,
            ):
                for nt in range(NT):
                    pso = psout_pool.tile([128, 1024], f32, name=f"pso{nt}", tag="pso")
                    for dh in range(2):
                        for p in range(PT):
                            nc.tensor.matmul(
                                pso[:, dh * 512 : (dh + 1) * 512],
                                lhsT=ctxt_sb[p][:, nt * 128 : (nt + 1) * 128],
                                rhs=wo_sb[p][:, dh * 512 : (dh + 1) * 512],
                                start=(p == 0),
                                stop=(p == PT - 1),
                            )
                    osb = outp.tile([128, 1024], f32, name=f"osb{nt}", tag="osb")
                    nc.vector.tensor_copy(osb[:], pso[:])
                    nc.sync.dma_start(
                        out=out_d[nt * 128 : (nt + 1) * 128, :], in_=osb[:]
                    )

    if not nc.is_finalized():
        nc.finalize()
    return nc


def _get_module():
    if "nc" not in _CACHE:
        _CACHE["nc"] = _build_module()
    return _CACHE["nc"]


def make_in_maps(x, W_q, W_k, W_v, W_o):
    in_maps = []
    for c in range(8):
        b, g = c // 2, c % 2
        gs = slice(g * 512, (g + 1) * 512)
        xT = np.ascontiguousarray(x[b].T).astype(BF16)
        wqkv = np.concatenate(
            [W_q[:, gs], W_k[:, gs], W_v[:, gs]], axis=1
        ).astype(BF16)
        wo = np.ascontiguousarray(W_o[gs, :]).astype(BF16)
        in_maps.append({"xT": xT, "wqkv": wqkv, "wo": wo})
    return in_maps


def kernel(x, W_q, W_k, W_v, W_o, b_o):
    from concourse.bass_utils import run_bass_kernel_spmd

    nc = _get_module()
    in_maps = make_in_maps(x, W_q, W_k, W_v, W_o)
    res = run_bass_kernel_spmd(nc, in_maps, core_ids=list(range(8)))

    out = np.empty((4, N_SEQ, D_EMB), np.float32)
    for b in range(4):
        out[b] = (
            res.results[2 * b]["out"]
            + res.results[2 * b + 1]["out"]
            + b_o[None, :].astype(np.float32)
        )
    return out


# revision 15
# speedup vs baseline: 267.5854x; 4.3029x over previous
"""Causal self-attention Trainium2 kernel (8 NeuronCores).

Sharding (Megatron-style, per sharding_hint):
  core c -> batch b = c//2, head-group g = c%2 (8 of 16 heads).
  W_q/W_k/W_v column-sliced per head group; W_o row-sliced; host sums the
  two partial outputs per batch (tensor-parallel reduce) and adds b_o.

Per-core kernel (all matmuls bf16 with fp32 PSUM accumulation):
  xT    [1024, 2048]  x[b] transposed (d_emb on partitions)
  wqkv  [1024, 1536]  [Wq_g | Wk_g | Wv_g]
  wo    [512, 1024]   W_o rows for this head group
  out   [2048, 1024]  fp32 partial (no bias)

Layouts: qT/kT stored [head_dim, n] so score matmuls contract over the
64-dim head axis; the two heads of a partition-tile occupy partitions
0:64 / 64:128, and their score matmuls are emitted interleaved so the PE
runs them concurrently in different row groups. Scores are computed
TRANSPOSED ([k, q]) so the exp'd weights feed the ctx matmul directly as
the moving operand; v is kept [n, head_dim] with a ones-block per head so
a single matmul yields both ctx^T and the softmax denominators broadcast
across 64 partitions.

Causal handling: k-tiles strictly above the diagonal are skipped; on
diagonal sub-tiles the fully-masked query prefix is never computed
(scores and ctx matmuls trim their moving operand to q >= k-block start),
and only the 128x128 diagonal block gets a triangular bf16 multiply.

`reps` repeats the whole body inside one NEFF — used only for timing
((T(n)-T(1))/(n-1) cancels dispatch overhead); the graded path is reps=1.
"""

import sys

import numpy as np

sys.path.insert(0, "/opt/trn_rl_repo")

import ml_dtypes

BF16 = ml_dtypes.bfloat16

D_EMB = 1024
N_SEQ = 2048
N_HEADS_CORE = 8  # heads per core
HD = 64  # head dim
KT = D_EMB // 128  # 8 k-tiles over d_emb
PT = 4  # partition tiles over the 512 per-core head dims
NT = N_SEQ // 128  # 16 n-tiles
QC = N_SEQ // 512  # 4 query chunks of 512
SCALE = 1.0 / np.sqrt(HD)

_CACHE = {}


def _emit_body(nc, tc, mybir, sfx, xT_d, wqkv_d, wo_d, out_d):
    f32 = mybir.dt.float32
    bf16 = mybir.dt.bfloat16

    with tc.tile_pool(name=f"persist{sfx}", bufs=1) as persist:
        wo_sb = [
            persist.tile([128, D_EMB], bf16, name=f"wo{p}{sfx}", tag=f"wo{p}")
            for p in range(PT)
        ]
        qt_sb = [
            persist.tile([128, N_SEQ], bf16, name=f"qt{p}{sfx}", tag=f"qt{p}")
            for p in range(PT)
        ]
        kt_sb = [
            persist.tile([128, N_SEQ], bf16, name=f"kt{p}{sfx}", tag=f"kt{p}")
            for p in range(PT)
        ]
        ctxt_sb = [
            persist.tile([128, N_SEQ], bf16, name=f"ctxt{p}{sfx}", tag=f"ctxt{p}")
            for p in range(PT)
        ]
        # v per n-tile [128, 1024]: head h -> cols h*128:h*128+64 = v_h,
        # cols h*128+64:h*128+128 = 1.0 (softmax denominator ones-trick)
        v_sb = [
            persist.tile([128, 1024], bf16, name=f"v{nt}{sfx}", tag=f"v{nt}")
            for nt in range(NT)
        ]
        tri_sb = persist.tile([128, 128], bf16, name=f"tri{sfx}", tag="tri")

        def vaug_ap(nt, h):
            return v_sb[nt][:, h * 128 : (h + 1) * 128]

        # ---- constants (gpsimd, no deps) ----
        for nt in range(NT):
            ones_view = v_sb[nt].rearrange("p (h c) -> p h c", h=N_HEADS_CORE)
            nc.gpsimd.memset(ones_view[:, :, 64:128], 1.0)
        # tri[k_local, q_local] = 1.0 if q_local >= k_local else 0
        nc.gpsimd.memset(tri_sb[:], 1.0)
        nc.gpsimd.affine_select(
            out=tri_sb[:],
            in_=tri_sb[:],
            compare_op=mybir.AluOpType.is_ge,
            fill=0.0,
            base=0,
            pattern=[[1, 128]],
            channel_multiplier=-1,
        )

        with tc.tile_pool(name=f"xw{sfx}", bufs=1) as xw_pool:
            xt_sb = [
                xw_pool.tile([128, N_SEQ], bf16, name=f"xt{k}{sfx}", tag=f"xt{k}")
                for k in range(KT)
            ]
            wqkv_sb = [
                xw_pool.tile([128, 1536], bf16, name=f"wqkv{k}{sfx}", tag=f"wqkv{k}")
                for k in range(KT)
            ]

            # ---- input DMA, split across queues for parallel load ----
            for k in range(KT):
                nc.sync.dma_start(
                    out=xt_sb[k][:], in_=xT_d[k * 128 : (k + 1) * 128, :]
                )
                nc.gpsimd.dma_start(
                    out=wqkv_sb[k][:], in_=wqkv_d[k * 128 : (k + 1) * 128, :]
                )
            for p in range(PT):
                nc.sync.dma_start(
                    out=wo_sb[p][:], in_=wo_d[p * 128 : (p + 1) * 128, :]
                )

            with tc.tile_pool(name=f"psq{sfx}", bufs=3, space="PSUM") as psq_pool:
                # ---- v = x @ Wv  ([n, 512] per n-tile) ----
                for nt in range(NT):
                    psv = psq_pool.tile(
                        [128, 512], f32, name=f"psv{nt}{sfx}", tag="psv"
                    )
                    for k in range(KT):
                        nc.tensor.matmul(
                            psv[:],
                            lhsT=xt_sb[k][:, nt * 128 : (nt + 1) * 128],
                            rhs=wqkv_sb[k][:, 1024:1536],
                            start=(k == 0),
                            stop=(k == KT - 1),
                        )
                    v_view = v_sb[nt].rearrange("p (h c) -> p h c", h=N_HEADS_CORE)
                    nc.vector.tensor_copy(
                        v_view[:, :, 0:64],
                        psv.rearrange("p (h c) -> p h c", h=N_HEADS_CORE),
                    )

                # ---- qT, kT = (x @ Wq)^T, (x @ Wk)^T  [hd, n] layout ----
                for qn in range(QC):
                    nsl = slice(qn * 512, (qn + 1) * 512)
                    for p in range(PT):
                        for which, dst in ((0, qt_sb), (1, kt_sb)):
                            ps = psq_pool.tile(
                                [128, 512],
                                f32,
                                name=f"psqk{p}_{qn}_{which}{sfx}",
                                tag="psqk",
                            )
                            col0 = which * 512 + p * 128
                            for k in range(KT):
                                nc.tensor.matmul(
                                    ps[:],
                                    lhsT=wqkv_sb[k][:, col0 : col0 + 128],
                                    rhs=xt_sb[k][:, nsl],
                                    start=(k == 0),
                                    stop=(k == KT - 1),
                                )
                            nc.vector.tensor_copy(dst[p][:, nsl], ps[:])

        # ---- attention (qc outer so out-proj can follow each chunk) ----
        with (
            tc.tile_pool(name=f"expp{sfx}", bufs=4) as expp,
            tc.tile_pool(name=f"rpool{sfx}", bufs=4) as rpool,
            tc.tile_pool(name=f"outp{sfx}", bufs=3) as outp,
            tc.tile_pool(name=f"pssc{sfx}", bufs=2, space="PSUM") as pssc_pool,
            tc.tile_pool(name=f"psctx{sfx}", bufs=1, space="PSUM") as psctx_pool,
            tc.tile_pool(name=f"psout{sfx}", bufs=1, space="PSUM") as psout_pool,
        ):
            for qc in range(QC):
                q0 = qc * 512
                nk = 4 * qc + 4  # causal: k-tiles 0..nk-1
                ngroups = nk // 2
                for p in range(PT):
                    # both heads' ctx in one 2-bank tile: h2 -> cols h2*512
                    ctx_ps = psctx_pool.tile(
                        [128, 1024], f32, name=f"ctx{p}_{qc}{sfx}", tag="ctx"
                    )
                    for gi in range(ngroups):
                        ps = [
                            pssc_pool.tile(
                                [128, 1024],
                                f32,
                                name=f"sc{p}_{qc}_{gi}_{h2}{sfx}",
                                tag="sc",
                            )
                            for h2 in range(2)
                        ]
                        ex = [
                            expp.tile(
                                [128, 1024],
                                bf16,
                                name=f"ex{p}_{qc}_{gi}_{h2}{sfx}",
                                tag="ex",
                            )
                            for h2 in range(2)
                        ]
                        # interleave heads so PE overlaps the row-group pairs
                        for j in range(2):
                            ki = 2 * gi + j
                            jj = ki - 4 * qc  # >=0 on diagonal sub-tiles
                            t0 = max(0, 128 * jj)  # masked-prefix trim
                            for h2 in range(2):
                                hb = h2 * 64
                                nc.tensor.matmul(
                                    ps[h2][:, j * 512 + t0 : (j + 1) * 512],
                                    lhsT=kt_sb[p][
                                        hb : hb + 64, ki * 128 : (ki + 1) * 128
                                    ],
                                    rhs=qt_sb[p][hb : hb + 64, q0 + t0 : q0 + 512],
                                    start=True,
                                    stop=True,
                                )
                        for h2 in range(2):
                            nc.scalar.activation(
                                ex[h2][:],
                                ps[h2][:],
                                mybir.ActivationFunctionType.Exp,
                                scale=float(SCALE),
                            )
                        for j in range(2):
                            ki = 2 * gi + j
                            jj = ki - 4 * qc
                            if jj >= 0:  # triangular block on the diagonal
                                blk = slice(
                                    j * 512 + 128 * jj, j * 512 + 128 * jj + 128
                                )
                                for h2 in range(2):
                                    nc.vector.tensor_mul(
                                        ex[h2][:, blk], ex[h2][:, blk], tri_sb[:]
                                    )
                        for j in range(2):
                            ki = 2 * gi + j
                            jj = ki - 4 * qc
                            t0 = max(0, 128 * jj)
                            for h2 in range(2):
                                h = 2 * p + h2
                                nc.tensor.matmul(
                                    ctx_ps[:, h2 * 512 + t0 : (h2 + 1) * 512],
                                    lhsT=vaug_ap(ki, h),
                                    rhs=ex[h2][:, j * 512 + t0 : (j + 1) * 512],
                                    start=(ki == 0),
                                    stop=(ki == nk - 1),
                                )
                    rec = rpool.tile(
                        [64, 1024], f32, name=f"rec{p}_{qc}{sfx}", tag="rec"
                    )
                    nc.vector.reciprocal(rec[:], ctx_ps[64:128, :])
                    for h2 in range(2):
                        nc.vector.tensor_mul(
                            ctxt_sb[p][h2 * 64 : h2 * 64 + 64, q0 : q0 + 512],
                            ctx_ps[0:64, h2 * 512 : (h2 + 1) * 512],
                            rec[:, h2 * 512 : (h2 + 1) * 512],
                        )

                # ---- out = ctx @ Wo for this chunk's n-tiles ----
                for nt in range(4 * qc, 4 * qc + 4):
                    pso = psout_pool.tile(
                        [128, 1024], f32, name=f"pso{nt}{sfx}", tag="pso"
                    )
                    for dh in range(2):
                        for p in range(PT):
                            nc.tensor.matmul(
                                pso[:, dh * 512 : (dh + 1) * 512],
                                lhsT=ctxt_sb[p][:, nt * 128 : (nt + 1) * 128],
                                rhs=wo_sb[p][:, dh * 512 : (dh + 1) * 512],
                                start=(p == 0),
                                stop=(p == PT - 1),
                            )
                    osb = outp.tile(
                        [128, 1024], f32, name=f"osb{nt}{sfx}", tag="osb"
                    )
                    nc.vector.tensor_copy(osb[:], pso[:])
                    nc.sync.dma_start(
                        out=out_d[nt * 128 : (nt + 1) * 128, :], in_=osb[:]
                    )


def _build_module(reps=1):
    import concourse.bacc as bacc
    import concourse.mybir as mybir
    import concourse.tile as tile

    f32 = mybir.dt.float32
    bf16 = mybir.dt.bfloat16

    nc = bacc.Bacc()
    xT_d = nc.dram_tensor("xT", [D_EMB, N_SEQ], bf16, kind="ExternalInput")
    wqkv_d = nc.dram_tensor("wqkv", [D_EMB, 1536], bf16, kind="ExternalInput")
    wo_d = nc.dram_tensor("wo", [512, D_EMB], bf16, kind="ExternalInput")
    out_d = nc.dram_tensor("out", [N_SEQ, D_EMB], f32, kind="ExternalOutput")

    with tile.TileContext(nc) as tc:
        for rep in range(reps):
            _emit_body(
                nc, tc, mybir, f"_r{rep}" if reps > 1 else "",
                xT_d, wqkv_d, wo_d, out_d,
            )

    if not nc.is_finalized():
        nc.finalize()
    return nc


def _get_module(reps=1):
    key = f"nc{reps}"
    if key not in _CACHE:
        _CACHE[key] = _build_module(reps)
    return _CACHE[key]


def make_in_maps(x, W_q, W_k, W_v, W_o):
    in_maps = []
    for c in range(8):
        b, g = c // 2, c % 2
        gs = slice(g * 512, (g + 1) * 512)
        xT = np.ascontiguousarray(x[b].T).astype(BF16)
        wqkv = np.concatenate(
            [W_q[:, gs], W_k[:, gs], W_v[:, gs]], axis=1
        ).astype(BF16)
        wo = np.ascontiguousarray(W_o[gs, :]).astype(BF16)
        in_maps.append({"xT": xT, "wqkv": wqkv, "wo": wo})
    return in_maps


def kernel(x, W_q, W_k, W_v, W_o, b_o):
    from concourse.bass_utils import run_bass_kernel_spmd

    nc = _get_module()
    in_maps = make_in_maps(x, W_q, W_k, W_v, W_o)
    res = run_bass_kernel_spmd(nc, in_maps, core_ids=list(range(8)))

    out = np.empty((4, N_SEQ, D_EMB), np.float32)
    for b in range(4):
        out[b] = (
            res.results[2 * b]["out"]
            + res.results[2 * b + 1]["out"]
            + b_o[None, :].astype(np.float32)
        )
    return out
